# revision 1
# baseline (speedup 1.0000x reference)
"""Trainium2 Bass kernel for ChannelAttention (B=16, C=512, H=W=64).

Math (per batch b):
    xf = x[b] reshaped [C, N], N = H*W = 4096
    q = Wq @ xf + bq            [64, N]
    k = Wk @ xf + bk            [64, N]
    v = Wv @ xf + bv            [64, N]
    energy = q @ k.T            [64, 64]   (contraction over N)
    attn = softmax(energy, -1)
    z = attn @ v                [64, N]
    out = Wo @ z + bo           [C, N]

Sharding: data-parallel over batch, 2 batches per core on 8 cores, no
collectives.  Each core receives its x shard plus the (host-pre-transposed)
weights and returns its out shard.

On-chip dataflow per batch (8 n-panels of 512), default scheme "b":
  - qT|kT projected DIRECTLY in transposed [n, q|k] layout: per 128-wide
    n-subtile, 4 accumulating matmuls with the xf c-chunk as the stationary
    operand (lhsT) and [WqT|WkT] as the moving operand.  This avoids any
    explicit transposes; biases are added along the free dim with a
    broadcast tile during the PSUM->SBUF copy on DVE.  The energy
    [64, 64] accumulates over all 32 n-subtiles as qT.T @ kT in full fp32
    (softmax is sensitive to energy error: values are ~N(0, 64^2), so the
    top-2 gap can be small and tf32-level error would be amplified).
  - v projected in native [64, n] layout (float32r, 1 cycle/row), kept in
    SBUF for the whole batch.
  - softmax: DVE row-max (negated), ACT exp with bias=-max and accum_out
    row-sum, DVE reciprocal + row scale of attn in place.
  - out = Wo @ (attn @ v) + bo is reassociated as (Wo @ attn) @ v:
    W2T = attn.T-free matmul (lhsT=attn native, rhs=WoT, one instruction),
    then out m-tiles = W2T-slice.T @ v panels in float32r, bias added on
    DVE during the PSUM->SBUF copy, DMA'd out per [128, 512] tile.

Matmul dtype notes: float32 is exact but runs at 4 cycles/row on the PE;
float32r runs at 1 cycle/row (for free dim >= 256) with ~tf32 multiply
precision (measured ~5e-4 relative on this problem's linear paths).
fp32r operands must be *typed* float32r at their producer (DMA from an
fp32r DRAM tensor, or an ACT/DVE copy with fp32r output); the bytes are
plain fp32 and can be bitcast back for exact fp32 consumers.
Measured on HW: rel L2 error 6.2e-4 vs the fp32 reference; ~150-180 us
per-core device time (2 batches/core), vs a ~70 us pure-DMA floor.
"""

import os

import numpy as np

# Problem shape (hardcoded; kernel.py must be self-contained).
B, C, H, W = 16, 512, 64, 64
N = H * W  # 4096
C8 = 64
P = 128
NCORES = 8
BPC = B // NCORES  # batches per core
CCH = C // P  # 4 c-chunks of 128
NP = 512  # n-panel width
NPANELS = N // NP  # 8
NSUB = NP // P  # 4 transpose subtiles per panel

# Matmul dtype knobs ("f32" = exact, "f32r" = fast single-pass).
QK_DT = os.environ.get("CHATT_QK_DT", "f32")
V_DT = os.environ.get("CHATT_V_DT", "f32r")
EN_DT = os.environ.get("CHATT_EN_DT", "f32")
ZO_DT = os.environ.get("CHATT_ZO_DT", "f32r")
# Timing aid: repeat the whole body REPS times inside a hardware loop so the
# device time is measurable above the host<->device transfer noise.
REPS = int(os.environ.get("CHATT_REPS", "1"))
# Bisection aids (timing experiments only; outputs become wrong):
SKIP_ENERGY = os.environ.get("CHATT_SKIP_ENERGY", "0") == "1"
SKIP_PHASEB = os.environ.get("CHATT_SKIP_PHASEB", "0") == "1"
# Energy-path structure:
#  "t": project q|k in native layout, PE-transpose panels, energy from qkT
#  "b": project qT|kT directly (xf chunks as stationary operand) - fewer
#       cross-engine hops, exact fp32 energy path, no transposes
SCHEME = os.environ.get("CHATT_SCHEME", "b")
# Engine for the out-tile PSUM->SBUF bias copies: "dve", "act", or "alt"
OUT_ENG = os.environ.get("CHATT_OUT_ENG", "dve")
# Out-DMA granularity: "mtile" = [128,512] per (mo,panel); "panel" = staged
# [512,512] per panel (fewer, bigger DMAs)
OUT_STAGE = os.environ.get("CHATT_OUT_STAGE", "mtile")
# Input DMA granularity: panels per dma_start (1 -> 1MB, 2 -> 2MB)
XF_PANELS = int(os.environ.get("CHATT_XF_PANELS", "1"))

_CACHE = {}
LAST_RESULTS = None


def _build_program():
    import concourse.bass as bass  # noqa: F401
    import concourse.mybir as mybir
    import concourse.tile as tile
    from concourse import bacc
    from concourse.masks import make_identity
    from contextlib import ExitStack

    f32 = mybir.dt.float32
    f32r = mybir.dt.float32r

    def dt_of(kind):
        return f32r if kind == "f32r" else f32

    # xf feeds both the qk and v projections; it is typed f32r if either
    # consumer is f32r, and bitcast back to f32 for an exact consumer
    # (fp32r bytes are fp32 bytes; the precision reduction happens in the PE).
    xf_dt = f32r if (QK_DT == "f32r" or V_DT == "f32r") else f32

    def x_cast(ap, kind):
        # cast xf slice to the dtype wanted by this matmul
        want = dt_of(kind)
        return ap if ap.dtype == want else ap.bitcast(want)

    nc = bacc.Bacc("TRN2", target_bir_lowering=False)

    x_h = nc.dram_tensor("x", [BPC, C, N], xf_dt, kind="ExternalInput")
    wqk_h = nc.dram_tensor("w_qkt", [C, P], dt_of(QK_DT), kind="ExternalInput")
    wv_h = nc.dram_tensor("w_vt", [C, C8], dt_of(V_DT), kind="ExternalInput")
    wo_h = nc.dram_tensor("w_ot", [C8, C], dt_of(ZO_DT), kind="ExternalInput")
    bqk_h = nc.dram_tensor("b_qk", [P], f32, kind="ExternalInput")
    bv_h = nc.dram_tensor("b_v", [C8], f32, kind="ExternalInput")
    bo_h = nc.dram_tensor("b_o", [C], f32, kind="ExternalInput")
    y_h = nc.dram_tensor("y", [BPC, C, N], f32, kind="ExternalOutput")

    x_ap = x_h.ap()
    y_ap = y_h.ap()

    with tile.TileContext(nc) as tc, ExitStack() as ctx:
        def _n(name, default):
            return int(os.environ.get(f"CHATT_BUFS_{name}", str(default)))

        consts = ctx.enter_context(tc.tile_pool(name="consts", bufs=1))
        xp = ctx.enter_context(
            tc.tile_pool(name="xp", bufs=_n("XP", max(2, 8 // XF_PANELS)))
        )
        qkp = ctx.enter_context(tc.tile_pool(name="qkp", bufs=_n("QKP", 3)))
        qktp = ctx.enter_context(tc.tile_pool(name="qktp", bufs=_n("QKTP", 4)))
        vp = ctx.enter_context(tc.tile_pool(name="vp", bufs=2))
        zp = ctx.enter_context(tc.tile_pool(name="zp", bufs=3))
        op = ctx.enter_context(
            tc.tile_pool(name="op", bufs=_n("OP", 6 if OUT_STAGE == "mtile" else 3))
        )
        smallp = ctx.enter_context(tc.tile_pool(name="smallp", bufs=4))
        # PSUM: 8 banks total.
        # scheme t: proj(qk+v) 3 + transpose 2 + energy 1 + out 2
        # scheme b: proj(v) 2 + qkT 3 + energy 1 + out 2
        ps_cfg = os.environ.get("CHATT_PSUM", "b" if SCHEME == "b" else "a")
        pe_n = 1
        if ps_cfg == "b":
            pp_n, pt_n, pzo_n = (2, 3, 2)
        elif ps_cfg == "e2":
            # double-buffer the energy bank so batch b+1's energy
            # accumulation doesn't wait for batch b's softmax
            pp_n, pt_n, pe_n, pzo_n = 2, 2, 2, 2
        else:
            pp_n, pt_n, pzo_n = (3, 2, 2)
        pp = ctx.enter_context(tc.tile_pool(name="pp", bufs=pp_n, space="PSUM"))
        pt = ctx.enter_context(tc.tile_pool(name="pt", bufs=pt_n, space="PSUM"))
        pe = ctx.enter_context(tc.tile_pool(name="pe", bufs=pe_n, space="PSUM"))
        pzo = ctx.enter_context(tc.tile_pool(name="pzo", bufs=pzo_n, space="PSUM"))

        # One-time constants.
        wqk_sb = consts.tile([P, CCH, P], dt_of(QK_DT))
        nc.sync.dma_start(wqk_sb, wqk_h.ap().rearrange("(co ci) m -> ci co m", ci=P))
        wv_sb = consts.tile([P, CCH, C8], dt_of(V_DT))
        nc.sync.dma_start(wv_sb, wv_h.ap().rearrange("(co ci) m -> ci co m", ci=P))
        wo_sb = consts.tile([C8, C], dt_of(ZO_DT))
        nc.sync.dma_start(wo_sb, wo_h.ap())
        bqk_sb = consts.tile([P, 1], f32)
        nc.sync.dma_start(bqk_sb, bqk_h.ap()[:, None])
        bv_sb = consts.tile([C8, 1], f32)
        nc.sync.dma_start(bv_sb, bv_h.ap()[:, None])
        bo_sb = consts.tile([P, CCH], f32)
        nc.sync.dma_start(bo_sb, bo_h.ap().rearrange("(mo mi) -> mi mo", mi=P))
        ident = consts.tile([P, P], f32)
        make_identity(nc, ident)
        if SCHEME == "b":
            # b_qk broadcast to all partitions: [128, 128] with the bias
            # along the free dim (for the transposed-layout bias add)
            bqk_bc = consts.tile([P, P], f32)
            nc.sync.dma_start(
                bqk_bc,
                bass.AP(tensor=bqk_h, offset=0, ap=[[0, P], [1, P]]),
            )

        Identity = mybir.ActivationFunctionType.Identity
        Copy = mybir.ActivationFunctionType.Copy
        Exp = mybir.ActivationFunctionType.Exp

        from contextlib import nullcontext

        hint = (
            (
                mybir.EngineType.PE,
                mybir.EngineType.Activation,
                mybir.EngineType.DVE,
                mybir.EngineType.SP,
            )
            if os.environ.get("CHATT_HINT", "0") == "1"
            else ()
        )
        rep_cm = (
            tc.For_i(0, REPS, 1, hint_engines=hint) if REPS > 1 else nullcontext()
        )
        with rep_cm:
            for b in range(BPC):
                xb = x_ap[b].rearrange("(co ci) n -> ci co n", ci=P)
                yb = y_ap[b].rearrange("(mo mi) n -> mi mo n", mi=P)

                energy = pe.tile([C8, C8], f32, tag="energy", name=f"energy_{b}")
                v_sb = vp.tile([C8, N], dt_of(ZO_DT), tag="v", name=f"v_{b}")

                # ---- Phase A: projections + energy accumulation ----
                xf_group = {}
                for p in range(NPANELS):
                    nsl = slice(p * NP, (p + 1) * NP)
                    if p % XF_PANELS == 0:
                        gw = XF_PANELS * NP
                        xf_g = xp.tile(
                            [P, CCH, gw], xf_dt, tag="xf", name=f"xf_{b}_{p}"
                        )
                        nc.sync.dma_start(
                            xf_g, xb[:, :, p * NP : p * NP + gw]
                        )
                        xf_group = {"tile": xf_g, "base": p}
                    off = (p - xf_group["base"]) * NP
                    xf = xf_group["tile"][:, :, off : off + NP]

                    v_ps = pp.tile([C8, NP], f32, tag="proj", name=f"vps_{b}_{p}")
                    for co in range(CCH):
                        nc.tensor.matmul(
                            v_ps,
                            wv_sb[:, co, :],
                            x_cast(xf[:, co, :], V_DT),
                            start=(co == 0),
                            stop=(co == CCH - 1),
                        )
                    nc.scalar.activation(
                        v_sb[:, nsl], v_ps, Identity, bias=bv_sb, scale=1.0
                    )

                    last_p = 0 if SKIP_ENERGY else NPANELS - 1
                    if SCHEME == "b":
                        if not (SKIP_ENERGY and p > 0):
                            for ns in range(NSUB):
                                qt_ps = pt.tile(
                                    [P, P], f32, tag="tp", name=f"qtps_{b}_{p}_{ns}"
                                )
                                for co in range(CCH):
                                    nc.tensor.matmul(
                                        qt_ps,
                                        x_cast(
                                            xf[:, co, ns * P : (ns + 1) * P], QK_DT
                                        ),
                                        wqk_sb[:, co, :],
                                        start=(co == 0),
                                        stop=(co == CCH - 1),
                                    )
                                qkt_sb = qktp.tile(
                                    [P, P],
                                    dt_of(EN_DT),
                                    tag="qkt",
                                    name=f"qkt_{b}_{p}_{ns}",
                                )
                                nc.vector.tensor_tensor(
                                    qkt_sb, qt_ps, bqk_bc, mybir.AluOpType.add
                                )
                                nc.tensor.matmul(
                                    energy,
                                    qkt_sb[:, 0:C8],
                                    qkt_sb[:, C8:P],
                                    start=(p == 0 and ns == 0),
                                    stop=(p == last_p and ns == NSUB - 1),
                                )
                    else:
                        qk_ps = pp.tile([P, NP], f32, tag="proj", name=f"qkps_{b}_{p}")
                        for co in range(CCH):
                            nc.tensor.matmul(
                                qk_ps,
                                wqk_sb[:, co, :],
                                x_cast(xf[:, co, :], QK_DT),
                                start=(co == 0),
                                stop=(co == CCH - 1),
                            )
                        qk_sb = qkp.tile([P, NP], f32, tag="qk", name=f"qk_{b}_{p}")
                        nc.scalar.activation(
                            qk_sb, qk_ps, Identity, bias=bqk_sb, scale=1.0
                        )
                        if not (SKIP_ENERGY and p > 0):
                            for ns in range(NSUB):
                                t_ps = pt.tile(
                                    [P, P], f32, tag="tp", name=f"tps_{b}_{p}_{ns}"
                                )
                                nc.tensor.transpose(
                                    t_ps, qk_sb[:, ns * P : (ns + 1) * P], ident
                                )
                                qkt_sb = qktp.tile(
                                    [P, P],
                                    dt_of(EN_DT),
                                    tag="qkt",
                                    name=f"qkt_{b}_{p}_{ns}",
                                )
                                nc.vector.tensor_copy(qkt_sb, t_ps)
                                nc.tensor.matmul(
                                    energy,
                                    qkt_sb[:, 0:C8],
                                    qkt_sb[:, C8:P],
                                    start=(p == 0 and ns == 0),
                                    stop=(p == last_p and ns == NSUB - 1),
                                )

                # ---- Phase B: softmax, W2 = Wo @ (attn/rowsum), out = W2 @ v
                negmax = smallp.tile([C8, 1], f32, tag="negmax", name=f"negmax_{b}")
                nc.vector.reduce_max(
                    negmax, energy, axis=mybir.AxisListType.X, negate=True
                )
                attn = smallp.tile([C8, C8], f32, tag="attn", name=f"attn_{b}")
                rowsum = smallp.tile([C8, 1], f32, tag="rowsum", name=f"rowsum_{b}")
                nc.scalar.activation(
                    attn, energy, Exp, bias=negmax, scale=1.0, accum_out=rowsum
                )
                recip = smallp.tile([C8, 1], f32, tag="recip", name=f"recip_{b}")
                nc.vector.reciprocal(recip, rowsum)
                # normalize attn rows in place (per-partition scale)
                nc.vector.tensor_scalar_mul(attn, attn, recip)

                # W2T[d, o] = sum_c attn[c, d] WoT[c, o]  (one matmul)
                w2_ps = pt.tile([C8, C], f32, tag="tp", name=f"w2ps_{b}")
                nc.tensor.matmul(w2_ps, attn, wo_sb.bitcast(f32), start=True, stop=True)
                w2_sb = zp.tile([C8, C], dt_of(ZO_DT), tag="z", name=f"w2_{b}")
                nc.vector.tensor_copy(w2_sb, w2_ps)

                if SKIP_PHASEB:
                    for p in range(NPANELS):
                        nsl = slice(p * NP, (p + 1) * NP)
                        nc.sync.dma_start(yb[:C8, 0, nsl], v_sb[:, nsl].bitcast(f32))
                    continue
                if OUT_STAGE == "panel":
                    for p in range(NPANELS):
                        nsl = slice(p * NP, (p + 1) * NP)
                        o_sb = op.tile(
                            [P, CCH, NP], f32, tag="o", name=f"o_{b}_{p}"
                        )
                        for mo in range(CCH):
                            o_ps = pzo.tile(
                                [P, NP], f32, tag="zo", name=f"ops_{b}_{p}_{mo}"
                            )
                            nc.tensor.matmul(
                                o_ps,
                                w2_sb[:, mo * P : (mo + 1) * P],
                                v_sb[:, nsl],
                                start=True,
                                stop=True,
                            )
                            use_act = OUT_ENG == "act" or (
                                OUT_ENG == "alt" and mo % 2 == 1
                            )
                            if use_act:
                                nc.scalar.activation(
                                    o_sb[:, mo, :], o_ps, Identity,
                                    bias=bo_sb[:, mo : mo + 1], scale=1.0,
                                )
                            else:
                                nc.vector.tensor_scalar_add(
                                    o_sb[:, mo, :], o_ps, bo_sb[:, mo : mo + 1]
                                )
                        nc.sync.dma_start(yb[:, :, nsl], o_sb)
                else:
                    for mo in range(CCH):
                        for p in range(NPANELS):
                            nsl = slice(p * NP, (p + 1) * NP)
                            o_ps = pzo.tile(
                                [P, NP], f32, tag="zo", name=f"ops_{b}_{p}_{mo}"
                            )
                            nc.tensor.matmul(
                                o_ps,
                                w2_sb[:, mo * P : (mo + 1) * P],
                                v_sb[:, nsl],
                                start=True,
                                stop=True,
                            )
                            o_sb = op.tile(
                                [P, NP], f32, tag="o", name=f"o_{b}_{p}_{mo}"
                            )
                            use_act = OUT_ENG == "act" or (
                                OUT_ENG == "alt" and p % 2 == 1
                            )
                            if use_act:
                                nc.scalar.activation(
                                    o_sb, o_ps, Identity,
                                    bias=bo_sb[:, mo : mo + 1], scale=1.0,
                                )
                            else:
                                nc.vector.tensor_scalar_add(
                                    o_sb, o_ps, bo_sb[:, mo : mo + 1]
                                )
                            nc.sync.dma_start(yb[:, mo, nsl], o_sb)

    nc.compile()
    return nc


def _get_program():
    key = (QK_DT, V_DT, EN_DT, ZO_DT, REPS)
    if key not in _CACHE:
        _CACHE[key] = _build_program()
    return _CACHE[key]


def _host_inputs(x, Wq, bq, Wk, bk, Wv, bv, Wo, bo):
    """Build the per-core input maps (host-side shard + weight transposes)."""
    x = np.ascontiguousarray(x, dtype=np.float32).reshape(B, C, N)
    w_qkt = np.ascontiguousarray(
        np.concatenate([Wq, Wk], axis=0).T.astype(np.float32)
    )  # [C, 128]
    w_vt = np.ascontiguousarray(Wv.T.astype(np.float32))  # [C, 64]
    w_ot = np.ascontiguousarray(Wo.T.astype(np.float32))  # [64, C]
    b_qk = np.ascontiguousarray(
        np.concatenate([bq, bk], axis=0).astype(np.float32)
    )  # [128]
    b_v = np.ascontiguousarray(bv.astype(np.float32))
    b_o = np.ascontiguousarray(bo.astype(np.float32))

    in_maps = []
    for i in range(NCORES):
        in_maps.append(
            {
                "x": np.ascontiguousarray(x[i * BPC : (i + 1) * BPC]),
                "w_qkt": w_qkt,
                "w_vt": w_vt,
                "w_ot": w_ot,
                "b_qk": b_qk,
                "b_v": b_v,
                "b_o": b_o,
            }
        )
    return in_maps


def kernel(**inputs):
    global LAST_RESULTS
    from concourse.bass_utils import run_bass_kernel_spmd

    nc = _get_program()
    in_maps = _host_inputs(**inputs)
    res = run_bass_kernel_spmd(nc, in_maps, core_ids=list(range(NCORES)))
    LAST_RESULTS = res
    out = np.concatenate([r["y"] for r in res.results], axis=0)
    return out.reshape(B, C, H, W).astype(np.float32)



# revision 2
# speedup vs baseline: 746.2575x; 746.2575x over previous
"""Trainium2 Bass kernel for ChannelAttention (B=16, C=512, H=W=64).

Math (per batch b):
    xf = x[b] reshaped [C, N], N = H*W = 4096
    q = Wq @ xf + bq            [64, N]
    k = Wk @ xf + bk            [64, N]
    v = Wv @ xf + bv            [64, N]
    energy = q @ k.T            [64, 64]   (contraction over N)
    attn = softmax(energy, -1)
    z = attn @ v                [64, N]
    out = Wo @ z + bo           [C, N]

Sharding: data-parallel over batch, 2 batches per core on 8 cores, no
collectives.  Each core receives its x shard plus the (host-pre-transposed)
weights and returns its out shard.

On-chip dataflow per batch (8 n-panels of 512), scheme "b":
  - qT|kT projected DIRECTLY in transposed [n, q|k] layout: per 128-wide
    n-subtile, 4 accumulating matmuls with the xf c-chunk as the stationary
    operand (lhsT) and [WqT|WkT] as the moving operand.  This avoids any
    explicit transposes; biases are added along the free dim with a
    broadcast tile during the PSUM->SBUF copy on DVE.  The energy
    [64, 64] accumulates over all 32 n-subtiles as qT.T @ kT in fp32 PSUM.
  - v projected in native [64, n] layout, kept in SBUF for the whole batch.
  - softmax: DVE row-max (negated), ACT exp with bias=-max and accum_out
    row-sum, DVE reciprocal + row scale of attn.
  - out = Wo @ (attn @ v) + bo is reassociated as (Wo @ attn) @ v:
    W2T = attn.T-free matmul (lhsT=attn, rhs=WoT, one instruction),
    then out m-tiles = W2T-slice.T @ v panels, bias added on DVE/ACT
    (alternating) during the PSUM->SBUF copy, DMA'd out per [128, 512] tile.

Matmul dtype: fp16 everywhere.  On the TRN2 PE, fp16 runs at 1 cycle/row
at ANY output free size (fp32 is 4 cycles/row; fp32r is 4 cycles/row below
256-wide free dim, which the transposed q/k projection [*,128] and energy
[*,64] matmuls can't reach).  fp16 keeps 10 mantissa bits (~tf32), which a
CPU bit-exact emulation of this pipeline puts at rel_l2 ~ 3.8e-3 overall —
bf16 (8 bits) fails the 2e-2 gate at ~2.9e-2 because energy errors get
amplified by softmax near-ties (energy ~ N(0, 64^2)).  x ships as fp16
(halves input DMA); y returns as fp16 and is upcast on host (halves output
DMA, adds ~2e-4 error).  All PSUM accumulation is fp32.
"""

import os

import numpy as np

# Problem shape (hardcoded; kernel.py must be self-contained).
B, C, H, W = 16, 512, 64, 64
N = H * W  # 4096
C8 = 64
P = 128
NCORES = 8
BPC = B // NCORES  # batches per core
CCH = C // P  # 4 c-chunks of 128
NP = 512  # n-panel width
NPANELS = N // NP  # 8
NSUB = NP // P  # 4 transpose subtiles per panel

# Matmul dtype knobs ("f32" exact 4c/row, "f32r" tf32-ish, "f16" 1c/row).
QK_DT = os.environ.get("CHATT_QK_DT", "f16")
V_DT = os.environ.get("CHATT_V_DT", "f16")
EN_DT = os.environ.get("CHATT_EN_DT", "f16")
ZO_DT = os.environ.get("CHATT_ZO_DT", "f16")
OUT_DT = os.environ.get("CHATT_OUT_DT", "f16")  # wire dtype of y
# Timing aid: repeat the whole body REPS times inside a hardware loop so the
# device time is measurable above the host<->device transfer noise.
REPS = int(os.environ.get("CHATT_REPS", "1"))
# Engine for the out-tile PSUM->SBUF bias copies: "dve", "act", or "alt"
OUT_ENG = os.environ.get("CHATT_OUT_ENG", "alt")
# Input DMA granularity: panels per dma_start (1 -> 512KB fp16)
XF_PANELS = int(os.environ.get("CHATT_XF_PANELS", "1"))

_CACHE = {}
LAST_RESULTS = None


def _np_of(kind):
    return np.float16 if kind == "f16" else np.float32


def _build_program():
    import concourse.bass as bass  # noqa: F401
    import concourse.mybir as mybir
    import concourse.tile as tile
    from concourse import bacc
    from contextlib import ExitStack

    f32 = mybir.dt.float32
    f32r = mybir.dt.float32r
    f16 = mybir.dt.float16

    def dt_of(kind):
        return {"f32": f32, "f32r": f32r, "f16": f16}[kind]

    # xf feeds both the qk and v projections from one SBUF tile; its dtype
    # must serve both consumers.  f16 requires both consumers f16; within
    # the 4-byte family, f32r bytes are f32 bytes so a bitcast suffices.
    if "f16" in (QK_DT, V_DT):
        assert QK_DT == V_DT == "f16", "f16 xf requires QK_DT == V_DT == f16"
        xf_dt = f16
    else:
        xf_dt = f32r if (QK_DT == "f32r" or V_DT == "f32r") else f32

    def x_cast(ap, kind):
        want = dt_of(kind)
        if ap.dtype == want:
            return ap
        return ap.bitcast(want)

    nc = bacc.Bacc("TRN2", target_bir_lowering=False)

    x_h = nc.dram_tensor("x", [BPC, C, N], xf_dt, kind="ExternalInput")
    wqk_h = nc.dram_tensor("w_qkt", [C, P], dt_of(QK_DT), kind="ExternalInput")
    wv_h = nc.dram_tensor("w_vt", [C, C8], dt_of(V_DT), kind="ExternalInput")
    wo_h = nc.dram_tensor("w_ot", [C8, C], dt_of(ZO_DT), kind="ExternalInput")
    bqk_h = nc.dram_tensor("b_qk", [P], f32, kind="ExternalInput")
    bv_h = nc.dram_tensor("b_v", [C8], f32, kind="ExternalInput")
    bo_h = nc.dram_tensor("b_o", [C], f32, kind="ExternalInput")
    y_h = nc.dram_tensor("y", [BPC, C, N], dt_of(OUT_DT), kind="ExternalOutput")

    x_ap = x_h.ap()
    y_ap = y_h.ap()

    with tile.TileContext(nc) as tc, ExitStack() as ctx:
        def _n(name, default):
            return int(os.environ.get(f"CHATT_BUFS_{name}", str(default)))

        consts = ctx.enter_context(tc.tile_pool(name="consts", bufs=1))
        xp = ctx.enter_context(
            tc.tile_pool(name="xp", bufs=_n("XP", max(2, 8 // XF_PANELS)))
        )
        qktp = ctx.enter_context(tc.tile_pool(name="qktp", bufs=_n("QKTP", 4)))
        vp = ctx.enter_context(tc.tile_pool(name="vp", bufs=2))
        zp = ctx.enter_context(tc.tile_pool(name="zp", bufs=3))
        op = ctx.enter_context(tc.tile_pool(name="op", bufs=_n("OP", 6)))
        smallp = ctx.enter_context(tc.tile_pool(name="smallp", bufs=4))
        # PSUM: 8 banks total: v-proj 2 + qkT 2 + energy 2 + out 2.
        # Energy is double-buffered so batch b+1's accumulation doesn't
        # wait for batch b's softmax to read it.
        ps_cfg = os.environ.get("CHATT_PSUM", "e2")
        if ps_cfg == "e2":
            pp_n, pt_n, pe_n, pzo_n = 2, 2, 2, 2
        else:
            pp_n, pt_n, pe_n, pzo_n = 2, 3, 1, 2
        pp = ctx.enter_context(tc.tile_pool(name="pp", bufs=pp_n, space="PSUM"))
        pt = ctx.enter_context(tc.tile_pool(name="pt", bufs=pt_n, space="PSUM"))
        pe = ctx.enter_context(tc.tile_pool(name="pe", bufs=pe_n, space="PSUM"))
        pzo = ctx.enter_context(tc.tile_pool(name="pzo", bufs=pzo_n, space="PSUM"))

        # One-time constants.
        wqk_sb = consts.tile([P, CCH, P], dt_of(QK_DT))
        nc.sync.dma_start(wqk_sb, wqk_h.ap().rearrange("(co ci) m -> ci co m", ci=P))
        wv_sb = consts.tile([P, CCH, C8], dt_of(V_DT))
        nc.sync.dma_start(wv_sb, wv_h.ap().rearrange("(co ci) m -> ci co m", ci=P))
        wo_sb = consts.tile([C8, C], dt_of(ZO_DT))
        nc.sync.dma_start(wo_sb, wo_h.ap())
        bv_sb = consts.tile([C8, 1], f32)
        nc.sync.dma_start(bv_sb, bv_h.ap()[:, None])
        bo_sb = consts.tile([P, CCH], f32)
        nc.sync.dma_start(bo_sb, bo_h.ap().rearrange("(mo mi) -> mi mo", mi=P))
        # b_qk broadcast to all partitions: [128, 128] with the bias
        # along the free dim (for the transposed-layout bias add)
        bqk_bc = consts.tile([P, P], f32)
        nc.sync.dma_start(
            bqk_bc,
            bass.AP(tensor=bqk_h, offset=0, ap=[[0, P], [1, P]]),
        )

        Identity = mybir.ActivationFunctionType.Identity
        Exp = mybir.ActivationFunctionType.Exp

        from contextlib import nullcontext

        rep_cm = tc.For_i(0, REPS, 1) if REPS > 1 else nullcontext()
        with rep_cm:
            for b in range(BPC):
                xb = x_ap[b].rearrange("(co ci) n -> ci co n", ci=P)
                yb = y_ap[b].rearrange("(mo mi) n -> mi mo n", mi=P)

                energy = pe.tile([C8, C8], f32, tag="energy", name=f"energy_{b}")
                v_sb = vp.tile([C8, N], dt_of(ZO_DT), tag="v", name=f"v_{b}")

                # ---- Phase A: projections + energy accumulation ----
                xf_group = {}
                for p in range(NPANELS):
                    nsl = slice(p * NP, (p + 1) * NP)
                    if p % XF_PANELS == 0:
                        gw = XF_PANELS * NP
                        xf_g = xp.tile(
                            [P, CCH, gw], xf_dt, tag="xf", name=f"xf_{b}_{p}"
                        )
                        nc.sync.dma_start(
                            xf_g, xb[:, :, p * NP : p * NP + gw]
                        )
                        xf_group = {"tile": xf_g, "base": p}
                    off = (p - xf_group["base"]) * NP
                    xf = xf_group["tile"][:, :, off : off + NP]

                    v_ps = pp.tile([C8, NP], f32, tag="proj", name=f"vps_{b}_{p}")
                    for co in range(CCH):
                        nc.tensor.matmul(
                            v_ps,
                            wv_sb[:, co, :],
                            x_cast(xf[:, co, :], V_DT),
                            start=(co == 0),
                            stop=(co == CCH - 1),
                        )
                    nc.scalar.activation(
                        v_sb[:, nsl], v_ps, Identity, bias=bv_sb, scale=1.0
                    )

                    for ns in range(NSUB):
                        qt_ps = pt.tile(
                            [P, P], f32, tag="tp", name=f"qtps_{b}_{p}_{ns}"
                        )
                        for co in range(CCH):
                            nc.tensor.matmul(
                                qt_ps,
                                x_cast(xf[:, co, ns * P : (ns + 1) * P], QK_DT),
                                wqk_sb[:, co, :],
                                start=(co == 0),
                                stop=(co == CCH - 1),
                            )
                        qkt_sb = qktp.tile(
                            [P, P],
                            dt_of(EN_DT),
                            tag="qkt",
                            name=f"qkt_{b}_{p}_{ns}",
                        )
                        nc.vector.tensor_tensor(
                            qkt_sb, qt_ps, bqk_bc, mybir.AluOpType.add
                        )
                        nc.tensor.matmul(
                            energy,
                            qkt_sb[:, 0:C8],
                            qkt_sb[:, C8:P],
                            start=(p == 0 and ns == 0),
                            stop=(p == NPANELS - 1 and ns == NSUB - 1),
                        )

                # ---- Phase B: softmax, W2 = Wo @ attn, out = W2 @ v ----
                negmax = smallp.tile([C8, 1], f32, tag="negmax", name=f"negmax_{b}")
                nc.vector.reduce_max(
                    negmax, energy, axis=mybir.AxisListType.X, negate=True
                )
                attn = smallp.tile([C8, C8], f32, tag="attn", name=f"attn_{b}")
                rowsum = smallp.tile([C8, 1], f32, tag="rowsum", name=f"rowsum_{b}")
                nc.scalar.activation(
                    attn, energy, Exp, bias=negmax, scale=1.0, accum_out=rowsum
                )
                recip = smallp.tile([C8, 1], f32, tag="recip", name=f"recip_{b}")
                nc.vector.reciprocal(recip, rowsum)
                # normalize attn rows (per-partition scale), emit matmul dtype
                attn_mm = smallp.tile([C8, C8], dt_of(ZO_DT), tag="attnm",
                                      name=f"attnm_{b}")
                nc.vector.tensor_scalar_mul(attn_mm, attn, recip)

                # W2T[d, o] = sum_c attn[c, d] WoT[c, o]  (one matmul)
                w2_ps = pt.tile([C8, C], f32, tag="tp", name=f"w2ps_{b}")
                nc.tensor.matmul(w2_ps, attn_mm, wo_sb, start=True, stop=True)
                w2_sb = zp.tile([C8, C], dt_of(ZO_DT), tag="z", name=f"w2_{b}")
                nc.vector.tensor_copy(w2_sb, w2_ps)

                for mo in range(CCH):
                    for p in range(NPANELS):
                        nsl = slice(p * NP, (p + 1) * NP)
                        o_ps = pzo.tile(
                            [P, NP], f32, tag="zo", name=f"ops_{b}_{p}_{mo}"
                        )
                        nc.tensor.matmul(
                            o_ps,
                            w2_sb[:, mo * P : (mo + 1) * P],
                            v_sb[:, nsl],
                            start=True,
                            stop=True,
                        )
                        o_sb = op.tile(
                            [P, NP], dt_of(OUT_DT), tag="o", name=f"o_{b}_{p}_{mo}"
                        )
                        use_act = OUT_ENG == "act" or (
                            OUT_ENG == "alt" and p % 2 == 1
                        )
                        if use_act:
                            nc.scalar.activation(
                                o_sb, o_ps, Identity,
                                bias=bo_sb[:, mo : mo + 1], scale=1.0,
                            )
                        else:
                            nc.vector.tensor_scalar_add(
                                o_sb, o_ps, bo_sb[:, mo : mo + 1]
                            )
                        nc.sync.dma_start(yb[:, mo, nsl], o_sb)

    nc.compile()
    return nc


def _get_program():
    key = (QK_DT, V_DT, EN_DT, ZO_DT, OUT_DT, REPS)
    if key not in _CACHE:
        _CACHE[key] = _build_program()
    return _CACHE[key]


def _host_inputs(x, Wq, bq, Wk, bk, Wv, bv, Wo, bo):
    """Build the per-core input maps (host-side shard + weight transposes)."""
    xf_np = np.float16 if (QK_DT == "f16" or V_DT == "f16") else np.float32
    x = np.ascontiguousarray(x, dtype=np.float32).reshape(B, C, N).astype(xf_np)
    w_qkt = np.ascontiguousarray(
        np.concatenate([Wq, Wk], axis=0).T.astype(_np_of(QK_DT))
    )  # [C, 128]
    w_vt = np.ascontiguousarray(Wv.T.astype(_np_of(V_DT)))  # [C, 64]
    w_ot = np.ascontiguousarray(Wo.T.astype(_np_of(ZO_DT)))  # [64, C]
    b_qk = np.ascontiguousarray(
        np.concatenate([bq, bk], axis=0).astype(np.float32)
    )  # [128]
    b_v = np.ascontiguousarray(bv.astype(np.float32))
    b_o = np.ascontiguousarray(bo.astype(np.float32))

    in_maps = []
    for i in range(NCORES):
        in_maps.append(
            {
                "x": np.ascontiguousarray(x[i * BPC : (i + 1) * BPC]),
                "w_qkt": w_qkt,
                "w_vt": w_vt,
                "w_ot": w_ot,
                "b_qk": b_qk,
                "b_v": b_v,
                "b_o": b_o,
            }
        )
    return in_maps


def kernel(**inputs):
    global LAST_RESULTS
    from concourse.bass_utils import run_bass_kernel_spmd

    nc = _get_program()
    in_maps = _host_inputs(**inputs)
    res = run_bass_kernel_spmd(nc, in_maps, core_ids=list(range(NCORES)))
    LAST_RESULTS = res
    out = np.concatenate([r["y"] for r in res.results], axis=0)
    return out.reshape(B, C, H, W).astype(np.float32)


# revision 10
# speedup vs baseline: 858.4944x; 1.1504x over previous
"""Trainium2 Bass kernel for ChannelAttention (B=16, C=512, H=W=64).

Math (per batch b):
    xf = x[b] reshaped [C, N], N = H*W = 4096
    q = Wq @ xf + bq            [64, N]
    k = Wk @ xf + bk            [64, N]
    v = Wv @ xf + bv            [64, N]
    energy = q @ k.T            [64, 64]   (contraction over N)
    attn = softmax(energy, -1)
    z = attn @ v                [64, N]
    out = Wo @ z + bo           [C, N]

Sharding: data-parallel over batch, 2 batches per core on 8 cores, no
collectives.  Each core receives its x shard plus the (host-pre-transposed)
weights and returns its out shard.

On-chip dataflow per batch (8 n-panels of 512), scheme "b":
  - qT|kT projected DIRECTLY in transposed [n, q|k] layout: per 128-wide
    n-subtile, 4 accumulating matmuls with the xf c-chunk as the stationary
    operand (lhsT) and [WqT|WkT] as the moving operand.  This avoids any
    explicit transposes; biases are added along the free dim with a
    broadcast tile during the PSUM->SBUF copy on DVE.  The energy
    [64, 64] accumulates over all 32 n-subtiles as qT.T @ kT in fp32 PSUM.
  - v projected in native [64, n] layout, kept in SBUF for the whole batch.
  - softmax: DVE row-max (negated), ACT exp with bias=-max and accum_out
    row-sum, DVE reciprocal + row scale of attn.
  - out = Wo @ (attn @ v) + bo is reassociated as (Wo @ attn) @ v:
    W2T = attn.T-free matmul (lhsT=attn, rhs=WoT, one instruction),
    then out m-tiles = W2T-slice.T @ v panels, bias added on DVE/ACT
    (alternating) during the PSUM->SBUF copy, DMA'd out per [128, 512] tile.

Matmul dtype: fp16 everywhere.  On the TRN2 PE, fp16 runs at 1 cycle/row
at ANY output free size (fp32 is 4 cycles/row; fp32r is 4 cycles/row below
256-wide free dim, which the transposed q/k projection [*,128] and energy
[*,64] matmuls can't reach).  fp16 keeps 10 mantissa bits (~tf32), which a
CPU bit-exact emulation of this pipeline puts at rel_l2 ~ 3.8e-3 overall —
bf16 (8 bits) fails the 2e-2 gate at ~2.9e-2 because energy errors get
amplified by softmax near-ties (energy ~ N(0, 64^2)).  x ships as fp16
(halves input DMA); y returns as fp16 and is upcast on host (halves output
DMA, adds ~2e-4 error).  All PSUM accumulation is fp32.
"""

import os

import numpy as np

# Problem shape (hardcoded; kernel.py must be self-contained).
B, C, H, W = 16, 512, 64, 64
N = H * W  # 4096
C8 = 64
P = 128
NCORES = 8
BPC = B // NCORES  # batches per core
CCH = C // P  # 4 c-chunks of 128
NP = 512  # n-panel width
NPANELS = N // NP  # 8
NSUB = NP // P  # 4 transpose subtiles per panel

# Matmul dtype knobs ("f32" exact 4c/row, "f32r" tf32-ish, "f16" 1c/row).
QK_DT = os.environ.get("CHATT_QK_DT", "f16")
V_DT = os.environ.get("CHATT_V_DT", "f16")
EN_DT = os.environ.get("CHATT_EN_DT", "f16")
ZO_DT = os.environ.get("CHATT_ZO_DT", "f16")
OUT_DT = os.environ.get("CHATT_OUT_DT", "f16")  # wire dtype of y
# Timing aid: repeat the whole body REPS times inside a hardware loop so the
# device time is measurable above the host<->device transfer noise.  The
# loop body holds UNROLL copies (the tile pipeline then overlaps phase B of
# one rep with phase A of the next; the For_i back-edge drains all engines,
# which costs ~9us, so amortize it).
REPS = int(os.environ.get("CHATT_REPS", "1"))
UNROLL = int(os.environ.get("CHATT_UNROLL", "4"))
# Engine split for the out-tile PSUM->SBUF bias copies: every 8 tiles,
# the first OUT_ACT8 go to ACT, the rest to DVE (DVE also does the qkt
# bias adds, so ACT gets the bigger share).
OUT_ACT8 = int(os.environ.get("CHATT_OUT_ACT8", "5"))
# Input DMA granularity: panels per dma_start (2 -> 1MB fp16)
XF_PANELS = int(os.environ.get("CHATT_XF_PANELS", "1"))
# Output DMA granularity: panels per dma_start (4 -> 512KB fp16, 4KB desc)
OUT_PANELS = int(os.environ.get("CHATT_OUT_PANELS", "4"))
# Issue output DMAs on the Activation HWDGE queue (input stays on SP)
OUT_QUEUE_ACT = os.environ.get("CHATT_OUT_QACT", "0") == "1"

_CACHE = {}
LAST_RESULTS = None


def _np_of(kind):
    return np.float16 if kind == "f16" else np.float32


def _build_program():
    import concourse.bass as bass  # noqa: F401
    import concourse.mybir as mybir
    import concourse.tile as tile
    from concourse import bacc
    from contextlib import ExitStack

    f32 = mybir.dt.float32
    f32r = mybir.dt.float32r
    f16 = mybir.dt.float16

    def dt_of(kind):
        return {"f32": f32, "f32r": f32r, "f16": f16}[kind]

    # xf feeds both the qk and v projections from one SBUF tile; its dtype
    # must serve both consumers.  f16 requires both consumers f16; within
    # the 4-byte family, f32r bytes are f32 bytes so a bitcast suffices.
    if "f16" in (QK_DT, V_DT):
        assert QK_DT == V_DT == "f16", "f16 xf requires QK_DT == V_DT == f16"
        xf_dt = f16
    else:
        xf_dt = f32r if (QK_DT == "f32r" or V_DT == "f32r") else f32

    def x_cast(ap, kind):
        want = dt_of(kind)
        if ap.dtype == want:
            return ap
        return ap.bitcast(want)

    nc = bacc.Bacc("TRN2", target_bir_lowering=False)

    x_h = nc.dram_tensor("x", [BPC, C, N], xf_dt, kind="ExternalInput")
    wqk_h = nc.dram_tensor("w_qkt", [C, P], dt_of(QK_DT), kind="ExternalInput")
    wv_h = nc.dram_tensor("w_vt", [C, C8], dt_of(V_DT), kind="ExternalInput")
    wo_h = nc.dram_tensor("w_ot", [C8, C], dt_of(ZO_DT), kind="ExternalInput")
    bqk_h = nc.dram_tensor("b_qk", [P], f32, kind="ExternalInput")
    bv_h = nc.dram_tensor("b_v", [C8], f32, kind="ExternalInput")
    bo_h = nc.dram_tensor("b_o", [C], f32, kind="ExternalInput")
    y_h = nc.dram_tensor("y", [BPC, C, N], dt_of(OUT_DT), kind="ExternalOutput")

    x_ap = x_h.ap()
    y_ap = y_h.ap()

    with tile.TileContext(nc) as tc, ExitStack() as ctx:
        def _n(name, default):
            return int(os.environ.get(f"CHATT_BUFS_{name}", str(default)))

        consts = ctx.enter_context(tc.tile_pool(name="consts", bufs=1))
        xp = ctx.enter_context(
            tc.tile_pool(name="xp", bufs=_n("XP", max(2, 8 // XF_PANELS)))
        )
        qktp = ctx.enter_context(tc.tile_pool(name="qktp", bufs=_n("QKTP", 4)))
        vp = ctx.enter_context(tc.tile_pool(name="vp", bufs=2))
        zp = ctx.enter_context(tc.tile_pool(name="zp", bufs=3))
        op = ctx.enter_context(tc.tile_pool(name="op", bufs=_n("OP", 4)))
        smallp = ctx.enter_context(tc.tile_pool(name="smallp", bufs=4))
        # PSUM: 8 banks total: v-proj 2 + qkT 2 + energy 1 + out 3.
        # Out gets 3 banks so the copy engines never wait on the
        # matmul->copy round trip; energy needs only 1 (softmax reads it
        # at phase-B start, long before the next batch's first energy
        # matmul needs the bank).
        ps_cfg = os.environ.get("CHATT_PSUM", "o3")
        if ps_cfg == "o3":
            pp_n, pt_n, pe_n, pzo_n = 2, 2, 1, 3
        elif ps_cfg == "e2":
            pp_n, pt_n, pe_n, pzo_n = 2, 2, 2, 2
        else:
            pp_n, pt_n, pe_n, pzo_n = 2, 3, 1, 2
        pp = ctx.enter_context(tc.tile_pool(name="pp", bufs=pp_n, space="PSUM"))
        pt = ctx.enter_context(tc.tile_pool(name="pt", bufs=pt_n, space="PSUM"))
        pe = ctx.enter_context(tc.tile_pool(name="pe", bufs=pe_n, space="PSUM"))
        pzo = ctx.enter_context(tc.tile_pool(name="pzo", bufs=pzo_n, space="PSUM"))

        # One-time constants.
        wqk_sb = consts.tile([P, CCH, P], dt_of(QK_DT))
        nc.sync.dma_start(wqk_sb, wqk_h.ap().rearrange("(co ci) m -> ci co m", ci=P))
        wv_sb = consts.tile([P, CCH, C8], dt_of(V_DT))
        nc.sync.dma_start(wv_sb, wv_h.ap().rearrange("(co ci) m -> ci co m", ci=P))
        wo_sb = consts.tile([C8, C], dt_of(ZO_DT))
        nc.sync.dma_start(wo_sb, wo_h.ap())
        bv_sb = consts.tile([C8, 1], f32)
        nc.sync.dma_start(bv_sb, bv_h.ap()[:, None])
        bo_sb = consts.tile([P, CCH], f32)
        nc.sync.dma_start(bo_sb, bo_h.ap().rearrange("(mo mi) -> mi mo", mi=P))
        # b_qk broadcast to all partitions: [128, 128] with the bias
        # along the free dim (for the transposed-layout bias add)
        bqk_bc = consts.tile([P, P], f32)
        nc.sync.dma_start(
            bqk_bc,
            bass.AP(tensor=bqk_h, offset=0, ap=[[0, P], [1, P]]),
        )

        Identity = mybir.ActivationFunctionType.Identity
        Exp = mybir.ActivationFunctionType.Exp

        from contextlib import nullcontext

        def emit_batch(u, b):
                xb = x_ap[b].rearrange("(co ci) n -> ci co n", ci=P)
                yb = y_ap[b].rearrange("(mo mi) n -> mi mo n", mi=P)

                energy = pe.tile([C8, C8], f32, tag="energy", name=f"energy_{u}_{b}")
                v_sb = vp.tile([C8, N], dt_of(ZO_DT), tag="v", name=f"v_{u}_{b}")

                # ---- Phase A: projections + energy accumulation ----
                xf_group = {}
                for p in range(NPANELS):
                    nsl = slice(p * NP, (p + 1) * NP)
                    if p % XF_PANELS == 0:
                        gw = XF_PANELS * NP
                        xf_g = xp.tile(
                            [P, CCH, gw], xf_dt, tag="xf", name=f"xf_{u}_{b}_{p}"
                        )
                        nc.sync.dma_start(
                            xf_g, xb[:, :, p * NP : p * NP + gw]
                        )
                        xf_group = {"tile": xf_g, "base": p}
                    off = (p - xf_group["base"]) * NP
                    xf = xf_group["tile"][:, :, off : off + NP]

                    v_ps = pp.tile([C8, NP], f32, tag="proj", name=f"vps_{u}_{b}_{p}")
                    for co in range(CCH):
                        nc.tensor.matmul(
                            v_ps,
                            wv_sb[:, co, :],
                            x_cast(xf[:, co, :], V_DT),
                            start=(co == 0),
                            stop=(co == CCH - 1),
                        )
                    nc.scalar.activation(
                        v_sb[:, nsl], v_ps, Identity, bias=bv_sb, scale=1.0
                    )

                    for ns in range(NSUB):
                        qt_ps = pt.tile(
                            [P, P], f32, tag="tp", name=f"qtps_{u}_{b}_{p}_{ns}"
                        )
                        for co in range(CCH):
                            nc.tensor.matmul(
                                qt_ps,
                                x_cast(xf[:, co, ns * P : (ns + 1) * P], QK_DT),
                                wqk_sb[:, co, :],
                                start=(co == 0),
                                stop=(co == CCH - 1),
                            )
                        qkt_sb = qktp.tile(
                            [P, P],
                            dt_of(EN_DT),
                            tag="qkt",
                            name=f"qkt_{u}_{b}_{p}_{ns}",
                        )
                        nc.vector.tensor_tensor(
                            qkt_sb, qt_ps, bqk_bc, mybir.AluOpType.add
                        )
                        nc.tensor.matmul(
                            energy,
                            qkt_sb[:, 0:C8],
                            qkt_sb[:, C8:P],
                            start=(p == 0 and ns == 0),
                            stop=(p == NPANELS - 1 and ns == NSUB - 1),
                        )

                # ---- Phase B: softmax, W2 = Wo @ attn, out = W2 @ v ----
                negmax = smallp.tile([C8, 1], f32, tag="negmax", name=f"negmax_{u}_{b}")
                nc.vector.reduce_max(
                    negmax, energy, axis=mybir.AxisListType.X, negate=True
                )
                attn = smallp.tile([C8, C8], f32, tag="attn", name=f"attn_{u}_{b}")
                rowsum = smallp.tile([C8, 1], f32, tag="rowsum", name=f"rowsum_{u}_{b}")
                nc.scalar.activation(
                    attn, energy, Exp, bias=negmax, scale=1.0, accum_out=rowsum
                )
                recip = smallp.tile([C8, 1], f32, tag="recip", name=f"recip_{u}_{b}")
                nc.vector.reciprocal(recip, rowsum)
                # normalize attn rows (per-partition scale), emit matmul dtype
                attn_mm = smallp.tile([C8, C8], dt_of(ZO_DT), tag="attnm",
                                      name=f"attnm_{u}_{b}")
                nc.vector.tensor_scalar_mul(attn_mm, attn, recip)

                # W2T[d, o] = sum_c attn[c, d] WoT[c, o]  (one matmul)
                w2_ps = pt.tile([C8, C], f32, tag="tp", name=f"w2ps_{u}_{b}")
                nc.tensor.matmul(w2_ps, attn_mm, wo_sb, start=True, stop=True)
                w2_sb = zp.tile([C8, C], dt_of(ZO_DT), tag="z", name=f"w2_{u}_{b}")
                nc.vector.tensor_copy(w2_sb, w2_ps)

                out_dma = nc.scalar if OUT_QUEUE_ACT else nc.sync
                tile_idx = 0
                for mo in range(CCH):
                    for pg in range(NPANELS // OUT_PANELS):
                        gw = OUT_PANELS * NP
                        o_sb = op.tile(
                            [P, gw], dt_of(OUT_DT), tag="o", name=f"o_{u}_{b}_{pg}_{mo}"
                        )
                        for pi in range(OUT_PANELS):
                            p = pg * OUT_PANELS + pi
                            nsl = slice(p * NP, (p + 1) * NP)
                            o_ps = pzo.tile(
                                [P, NP], f32, tag="zo", name=f"ops_{u}_{b}_{p}_{mo}"
                            )
                            nc.tensor.matmul(
                                o_ps,
                                w2_sb[:, mo * P : (mo + 1) * P],
                                v_sb[:, nsl],
                                start=True,
                                stop=True,
                            )
                            osl = slice(pi * NP, (pi + 1) * NP)
                            if tile_idx % 8 < OUT_ACT8:
                                nc.scalar.activation(
                                    o_sb[:, osl], o_ps, Identity,
                                    bias=bo_sb[:, mo : mo + 1], scale=1.0,
                                )
                            else:
                                nc.vector.tensor_scalar_add(
                                    o_sb[:, osl], o_ps, bo_sb[:, mo : mo + 1]
                                )
                            tile_idx += 1
                        out_dma.dma_start(
                            yb[:, mo, pg * gw : (pg + 1) * gw], o_sb
                        )

        if REPS > 1:
            u_total = UNROLL if REPS % UNROLL == 0 else 1
            with tc.For_i(0, REPS // u_total, 1):
                for u in range(u_total):
                    for b in range(BPC):
                        emit_batch(u, b)
        else:
            for b in range(BPC):
                emit_batch(0, b)

    nc.compile()
    return nc


def _get_program():
    key = (QK_DT, V_DT, EN_DT, ZO_DT, OUT_DT, REPS, UNROLL,
           OUT_ACT8, XF_PANELS, OUT_PANELS, OUT_QUEUE_ACT)
    if key not in _CACHE:
        _CACHE[key] = _build_program()
    return _CACHE[key]


def _host_inputs(x, Wq, bq, Wk, bk, Wv, bv, Wo, bo):
    """Build the per-core input maps (host-side shard + weight transposes)."""
    xf_np = np.float16 if (QK_DT == "f16" or V_DT == "f16") else np.float32
    x = np.ascontiguousarray(x, dtype=np.float32).reshape(B, C, N).astype(xf_np)
    w_qkt = np.ascontiguousarray(
        np.concatenate([Wq, Wk], axis=0).T.astype(_np_of(QK_DT))
    )  # [C, 128]
    w_vt = np.ascontiguousarray(Wv.T.astype(_np_of(V_DT)))  # [C, 64]
    w_ot = np.ascontiguousarray(Wo.T.astype(_np_of(ZO_DT)))  # [64, C]
    b_qk = np.ascontiguousarray(
        np.concatenate([bq, bk], axis=0).astype(np.float32)
    )  # [128]
    b_v = np.ascontiguousarray(bv.astype(np.float32))
    b_o = np.ascontiguousarray(bo.astype(np.float32))

    in_maps = []
    for i in range(NCORES):
        in_maps.append(
            {
                "x": np.ascontiguousarray(x[i * BPC : (i + 1) * BPC]),
                "w_qkt": w_qkt,
                "w_vt": w_vt,
                "w_ot": w_ot,
                "b_qk": b_qk,
                "b_v": b_v,
                "b_o": b_o,
            }
        )
    return in_maps


def kernel(**inputs):
    global LAST_RESULTS
    from concourse.bass_utils import run_bass_kernel_spmd

    nc = _get_program()
    in_maps = _host_inputs(**inputs)
    res = run_bass_kernel_spmd(nc, in_maps, core_ids=list(range(NCORES)))
    LAST_RESULTS = res
    out = np.concatenate([r["y"] for r in res.results], axis=0)
    return out.reshape(B, C, H, W).astype(np.float32)


# revision 18
# speedup vs baseline: 866.2355x; 1.0090x over previous
"""Trainium2 Bass kernel for ChannelAttention (B=16, C=512, H=W=64).

Math (per batch b):
    xf = x[b] reshaped [C, N], N = H*W = 4096
    q = Wq @ xf + bq            [64, N]
    k = Wk @ xf + bk            [64, N]
    v = Wv @ xf + bv            [64, N]
    energy = q @ k.T            [64, 64]   (contraction over N)
    attn = softmax(energy, -1)
    z = attn @ v                [64, N]
    out = Wo @ z + bo           [C, N]

Sharding: data-parallel over batch, 2 batches per core on 8 cores, no
collectives.  Each core receives its x shard plus the (host-pre-transposed)
weights and returns its out shard.

On-chip dataflow per batch (8 n-panels of 512), scheme "b":
  - qT|kT projected DIRECTLY in transposed [n, q|k] layout: per 128-wide
    n-subtile, 4 accumulating matmuls with the xf c-chunk as the stationary
    operand (lhsT) and [WqT|WkT] as the moving operand.  This avoids any
    explicit transposes; biases are added along the free dim with a
    broadcast tile during the PSUM->SBUF copy on DVE.  The energy
    [64, 64] accumulates over all 32 n-subtiles as qT.T @ kT in fp32 PSUM.
  - v projected in native [64, n] layout, kept in SBUF for the whole batch.
  - softmax: DVE row-max (negated), ACT exp with bias=-max and accum_out
    row-sum, DVE reciprocal + row scale of attn.
  - out = Wo @ (attn @ v) + bo is reassociated as (Wo @ attn) @ v:
    W2T = attn.T-free matmul (lhsT=attn, rhs=WoT, one instruction),
    then out m-tiles = W2T-slice.T @ v panels, bias added on DVE/ACT
    (alternating) during the PSUM->SBUF copy, DMA'd out per [128, 512] tile.

Matmul dtype: fp16 everywhere.  On the TRN2 PE, fp16 runs at 1 cycle/row
at ANY output free size (fp32 is 4 cycles/row; fp32r is 4 cycles/row below
256-wide free dim, which the transposed q/k projection [*,128] and energy
[*,64] matmuls can't reach).  fp16 keeps 10 mantissa bits (~tf32), which a
CPU bit-exact emulation of this pipeline puts at rel_l2 ~ 3.8e-3 overall —
bf16 (8 bits) fails the 2e-2 gate at ~2.9e-2 because energy errors get
amplified by softmax near-ties (energy ~ N(0, 64^2)).  x ships as fp16
(halves input DMA); y returns as fp16 and is upcast on host (halves output
DMA, adds ~2e-4 error).  All PSUM accumulation is fp32.
"""

import os

import numpy as np

# Problem shape (hardcoded; kernel.py must be self-contained).
B, C, H, W = 16, 512, 64, 64
N = H * W  # 4096
C8 = 64
P = 128
NCORES = 8
BPC = B // NCORES  # batches per core
CCH = C // P  # 4 c-chunks of 128
NP = 512  # n-panel width
NPANELS = N // NP  # 8
NSUB = NP // P  # 4 transpose subtiles per panel

# Matmul dtype knobs ("f32" exact 4c/row, "f32r" tf32-ish, "f16" 1c/row).
QK_DT = os.environ.get("CHATT_QK_DT", "f16")
V_DT = os.environ.get("CHATT_V_DT", "f16")
EN_DT = os.environ.get("CHATT_EN_DT", "f16")
ZO_DT = os.environ.get("CHATT_ZO_DT", "f16")
OUT_DT = os.environ.get("CHATT_OUT_DT", "f16")  # wire dtype of y
# Timing aid: repeat the whole body REPS times inside a hardware loop so the
# device time is measurable above the host<->device transfer noise.  The
# loop body holds UNROLL copies (the tile pipeline then overlaps phase B of
# one rep with phase A of the next; the For_i back-edge drains all engines,
# which costs ~9us, so amortize it).
REPS = int(os.environ.get("CHATT_REPS", "1"))
UNROLL = int(os.environ.get("CHATT_UNROLL", "4"))
# Engine split for the out-tile PSUM->SBUF bias copies: every 8 tiles,
# the first OUT_ACT8 go to ACT, the rest to DVE (DVE also does the qkt
# bias adds, so ACT gets the bigger share).
OUT_ACT8 = int(os.environ.get("CHATT_OUT_ACT8", "5"))
# Input DMA granularity: panels per dma_start (2 -> 1MB fp16)
XF_PANELS = int(os.environ.get("CHATT_XF_PANELS", "1"))
# Output DMA granularity: panels per dma_start (4 -> 512KB fp16, 4KB desc)
OUT_PANELS = int(os.environ.get("CHATT_OUT_PANELS", "4"))
# Issue output DMAs on the Activation HWDGE queue (input stays on SP)
OUT_QUEUE_ACT = os.environ.get("CHATT_OUT_QACT", "0") == "1"

_CACHE = {}
LAST_RESULTS = None


def _np_of(kind):
    return np.float16 if kind == "f16" else np.float32


def _build_program():
    import concourse.bass as bass  # noqa: F401
    import concourse.mybir as mybir
    import concourse.tile as tile
    from concourse import bacc
    from contextlib import ExitStack

    f32 = mybir.dt.float32
    f32r = mybir.dt.float32r
    f16 = mybir.dt.float16

    def dt_of(kind):
        return {"f32": f32, "f32r": f32r, "f16": f16}[kind]

    # xf feeds both the qk and v projections from one SBUF tile; its dtype
    # must serve both consumers.  f16 requires both consumers f16; within
    # the 4-byte family, f32r bytes are f32 bytes so a bitcast suffices.
    if "f16" in (QK_DT, V_DT):
        assert QK_DT == V_DT == "f16", "f16 xf requires QK_DT == V_DT == f16"
        xf_dt = f16
    else:
        xf_dt = f32r if (QK_DT == "f32r" or V_DT == "f32r") else f32

    def x_cast(ap, kind):
        want = dt_of(kind)
        if ap.dtype == want:
            return ap
        return ap.bitcast(want)

    nc = bacc.Bacc("TRN2", target_bir_lowering=False)

    # Wire layouts are chosen so every big DMA reads/writes one CONTIGUOUS
    # DRAM block (the host repacks): x is panel-group-major
    # [b, group, C, XF_PANELS*NP] so a group load is one sequential block
    # with XF_PANELS*1KB descriptor runs; y is tile-major
    # [b, mo, pgroup, mi, gw] so an output store is a sequential 512KB write.
    NGI = NPANELS // XF_PANELS
    GWI = XF_PANELS * NP
    x_h = nc.dram_tensor("x", [BPC, NGI, C, GWI], xf_dt, kind="ExternalInput")
    wqk_h = nc.dram_tensor("w_qkt", [C, P], dt_of(QK_DT), kind="ExternalInput")
    wv_h = nc.dram_tensor("w_vt", [C, C8], dt_of(V_DT), kind="ExternalInput")
    wo_h = nc.dram_tensor("w_ot", [C8, C], dt_of(ZO_DT), kind="ExternalInput")
    bqk_h = nc.dram_tensor("b_qk", [P], f32, kind="ExternalInput")
    bv_h = nc.dram_tensor("b_v", [C8], f32, kind="ExternalInput")
    bo_h = nc.dram_tensor("b_o", [C], f32, kind="ExternalInput")
    NGRP = NPANELS // OUT_PANELS
    GW = OUT_PANELS * NP
    y_h = nc.dram_tensor(
        "y", [BPC, CCH, NGRP, P, GW], dt_of(OUT_DT), kind="ExternalOutput"
    )

    x_ap = x_h.ap()
    y_ap = y_h.ap()

    with tile.TileContext(nc) as tc, ExitStack() as ctx:
        def _n(name, default):
            return int(os.environ.get(f"CHATT_BUFS_{name}", str(default)))

        consts = ctx.enter_context(tc.tile_pool(name="consts", bufs=1))
        xp = ctx.enter_context(
            tc.tile_pool(name="xp", bufs=_n("XP", max(2, 8 // XF_PANELS)))
        )
        qktp = ctx.enter_context(tc.tile_pool(name="qktp", bufs=_n("QKTP", 4)))
        vp = ctx.enter_context(tc.tile_pool(name="vp", bufs=2))
        zp = ctx.enter_context(tc.tile_pool(name="zp", bufs=3))
        op = ctx.enter_context(tc.tile_pool(name="op", bufs=_n("OP", 4)))
        smallp = ctx.enter_context(tc.tile_pool(name="smallp", bufs=4))
        # PSUM: 8 banks total: v-proj 2 + qkT 2 + energy 1 + out 3.
        # Out gets 3 banks so the copy engines never wait on the
        # matmul->copy round trip; energy needs only 1 (softmax reads it
        # at phase-B start, long before the next batch's first energy
        # matmul needs the bank).
        ps_cfg = os.environ.get("CHATT_PSUM", "o3")
        if ps_cfg == "o3":
            pp_n, pt_n, pe_n, pzo_n = 2, 2, 1, 3
        elif ps_cfg == "e2":
            pp_n, pt_n, pe_n, pzo_n = 2, 2, 2, 2
        else:
            pp_n, pt_n, pe_n, pzo_n = 2, 3, 1, 2
        pp = ctx.enter_context(tc.tile_pool(name="pp", bufs=pp_n, space="PSUM"))
        pt = ctx.enter_context(tc.tile_pool(name="pt", bufs=pt_n, space="PSUM"))
        pe = ctx.enter_context(tc.tile_pool(name="pe", bufs=pe_n, space="PSUM"))
        pzo = ctx.enter_context(tc.tile_pool(name="pzo", bufs=pzo_n, space="PSUM"))

        # One-time constants.
        wqk_sb = consts.tile([P, CCH, P], dt_of(QK_DT))
        nc.sync.dma_start(wqk_sb, wqk_h.ap().rearrange("(co ci) m -> ci co m", ci=P))
        wv_sb = consts.tile([P, CCH, C8], dt_of(V_DT))
        nc.sync.dma_start(wv_sb, wv_h.ap().rearrange("(co ci) m -> ci co m", ci=P))
        wo_sb = consts.tile([C8, C], dt_of(ZO_DT))
        nc.sync.dma_start(wo_sb, wo_h.ap())
        bv_sb = consts.tile([C8, 1], f32)
        nc.sync.dma_start(bv_sb, bv_h.ap()[:, None])
        bo_sb = consts.tile([P, CCH], f32)
        nc.sync.dma_start(bo_sb, bo_h.ap().rearrange("(mo mi) -> mi mo", mi=P))
        # b_qk broadcast to all partitions: [128, 128] with the bias
        # along the free dim (for the transposed-layout bias add)
        bqk_bc = consts.tile([P, P], f32)
        nc.sync.dma_start(
            bqk_bc,
            bass.AP(tensor=bqk_h, offset=0, ap=[[0, P], [1, P]]),
        )

        Identity = mybir.ActivationFunctionType.Identity
        Exp = mybir.ActivationFunctionType.Exp

        from contextlib import nullcontext

        def emit_batch(u, b):
                xb = x_ap[b]  # [NPANELS, C, NP]
                yb = y_ap[b]  # [CCH, NGRP, P, GW]

                energy = pe.tile([C8, C8], f32, tag="energy", name=f"energy_{u}_{b}")
                v_sb = vp.tile([C8, N], dt_of(ZO_DT), tag="v", name=f"v_{u}_{b}")

                # ---- Phase A: projections + energy accumulation ----
                xf_group = {}
                for p in range(NPANELS):
                    nsl = slice(p * NP, (p + 1) * NP)
                    if p % XF_PANELS == 0:
                        xf_g = xp.tile(
                            [P, CCH, XF_PANELS, NP], xf_dt, tag="xf",
                            name=f"xf_{u}_{b}_{p}",
                        )
                        nc.sync.dma_start(
                            xf_g,
                            xb[p : p + XF_PANELS].rearrange(
                                "g (co ci) np -> ci co g np", ci=P
                            ),
                        )
                        xf_group = {"tile": xf_g, "base": p}
                    gi = p - xf_group["base"]
                    xf = xf_group["tile"][:, :, gi, :]

                    v_ps = pp.tile([C8, NP], f32, tag="proj", name=f"vps_{u}_{b}_{p}")
                    for co in range(CCH):
                        nc.tensor.matmul(
                            v_ps,
                            wv_sb[:, co, :],
                            x_cast(xf[:, co, :], V_DT),
                            start=(co == 0),
                            stop=(co == CCH - 1),
                        )
                    nc.scalar.activation(
                        v_sb[:, nsl], v_ps, Identity, bias=bv_sb, scale=1.0
                    )

                    for ns in range(NSUB):
                        qt_ps = pt.tile(
                            [P, P], f32, tag="tp", name=f"qtps_{u}_{b}_{p}_{ns}"
                        )
                        for co in range(CCH):
                            nc.tensor.matmul(
                                qt_ps,
                                x_cast(xf[:, co, ns * P : (ns + 1) * P], QK_DT),
                                wqk_sb[:, co, :],
                                start=(co == 0),
                                stop=(co == CCH - 1),
                            )
                        qkt_sb = qktp.tile(
                            [P, P],
                            dt_of(EN_DT),
                            tag="qkt",
                            name=f"qkt_{u}_{b}_{p}_{ns}",
                        )
                        nc.vector.tensor_tensor(
                            qkt_sb, qt_ps, bqk_bc, mybir.AluOpType.add
                        )
                        nc.tensor.matmul(
                            energy,
                            qkt_sb[:, 0:C8],
                            qkt_sb[:, C8:P],
                            start=(p == 0 and ns == 0),
                            stop=(p == NPANELS - 1 and ns == NSUB - 1),
                        )

                # ---- Phase B: softmax, W2 = Wo @ attn, out = W2 @ v ----
                negmax = smallp.tile([C8, 1], f32, tag="negmax", name=f"negmax_{u}_{b}")
                nc.vector.reduce_max(
                    negmax, energy, axis=mybir.AxisListType.X, negate=True
                )
                attn = smallp.tile([C8, C8], f32, tag="attn", name=f"attn_{u}_{b}")
                rowsum = smallp.tile([C8, 1], f32, tag="rowsum", name=f"rowsum_{u}_{b}")
                nc.scalar.activation(
                    attn, energy, Exp, bias=negmax, scale=1.0, accum_out=rowsum
                )
                recip = smallp.tile([C8, 1], f32, tag="recip", name=f"recip_{u}_{b}")
                nc.vector.reciprocal(recip, rowsum)
                # normalize attn rows (per-partition scale), emit matmul dtype
                attn_mm = smallp.tile([C8, C8], dt_of(ZO_DT), tag="attnm",
                                      name=f"attnm_{u}_{b}")
                nc.vector.tensor_scalar_mul(attn_mm, attn, recip)

                # W2T[d, o] = sum_c attn[c, d] WoT[c, o]  (one matmul)
                w2_ps = pt.tile([C8, C], f32, tag="tp", name=f"w2ps_{u}_{b}")
                nc.tensor.matmul(w2_ps, attn_mm, wo_sb, start=True, stop=True)
                w2_sb = zp.tile([C8, C], dt_of(ZO_DT), tag="z", name=f"w2_{u}_{b}")
                nc.vector.tensor_copy(w2_sb, w2_ps)

                out_dma = nc.scalar if OUT_QUEUE_ACT else nc.sync
                tile_idx = 0
                for mo in range(CCH):
                    for pg in range(NGRP):
                        o_sb = op.tile(
                            [P, GW], dt_of(OUT_DT), tag="o", name=f"o_{u}_{b}_{pg}_{mo}"
                        )
                        for pi in range(OUT_PANELS):
                            p = pg * OUT_PANELS + pi
                            nsl = slice(p * NP, (p + 1) * NP)
                            o_ps = pzo.tile(
                                [P, NP], f32, tag="zo", name=f"ops_{u}_{b}_{p}_{mo}"
                            )
                            nc.tensor.matmul(
                                o_ps,
                                w2_sb[:, mo * P : (mo + 1) * P],
                                v_sb[:, nsl],
                                start=True,
                                stop=True,
                            )
                            osl = slice(pi * NP, (pi + 1) * NP)
                            if tile_idx % 8 < OUT_ACT8:
                                nc.scalar.activation(
                                    o_sb[:, osl], o_ps, Identity,
                                    bias=bo_sb[:, mo : mo + 1], scale=1.0,
                                )
                            else:
                                nc.vector.tensor_scalar_add(
                                    o_sb[:, osl], o_ps, bo_sb[:, mo : mo + 1]
                                )
                            tile_idx += 1
                        out_dma.dma_start(yb[mo, pg], o_sb)

        if REPS > 1:
            u_total = UNROLL if REPS % UNROLL == 0 else 1
            with tc.For_i(0, REPS // u_total, 1):
                for u in range(u_total):
                    for b in range(BPC):
                        emit_batch(u, b)
        else:
            for b in range(BPC):
                emit_batch(0, b)

    nc.compile()
    return nc


def _get_program():
    key = (QK_DT, V_DT, EN_DT, ZO_DT, OUT_DT, REPS, UNROLL,
           OUT_ACT8, XF_PANELS, OUT_PANELS, OUT_QUEUE_ACT)
    if key not in _CACHE:
        _CACHE[key] = _build_program()
    return _CACHE[key]


def _host_inputs(x, Wq, bq, Wk, bk, Wv, bv, Wo, bo):
    """Build the per-core input maps (host-side shard + weight transposes)."""
    xf_np = np.float16 if (QK_DT == "f16" or V_DT == "f16") else np.float32
    # panel-major wire layout: [B, NPANELS, C, NP], each panel contiguous
    x = (
        np.asarray(x, dtype=np.float32)
        .reshape(B, C, NPANELS, NP)
        .transpose(0, 2, 1, 3)
        .astype(xf_np)
    )
    w_qkt = np.ascontiguousarray(
        np.concatenate([Wq, Wk], axis=0).T.astype(_np_of(QK_DT))
    )  # [C, 128]
    w_vt = np.ascontiguousarray(Wv.T.astype(_np_of(V_DT)))  # [C, 64]
    w_ot = np.ascontiguousarray(Wo.T.astype(_np_of(ZO_DT)))  # [64, C]
    b_qk = np.ascontiguousarray(
        np.concatenate([bq, bk], axis=0).astype(np.float32)
    )  # [128]
    b_v = np.ascontiguousarray(bv.astype(np.float32))
    b_o = np.ascontiguousarray(bo.astype(np.float32))

    in_maps = []
    for i in range(NCORES):
        in_maps.append(
            {
                "x": np.ascontiguousarray(x[i * BPC : (i + 1) * BPC]),
                "w_qkt": w_qkt,
                "w_vt": w_vt,
                "w_ot": w_ot,
                "b_qk": b_qk,
                "b_v": b_v,
                "b_o": b_o,
            }
        )
    return in_maps


def kernel(**inputs):
    global LAST_RESULTS
    from concourse.bass_utils import run_bass_kernel_spmd

    nc = _get_program()
    in_maps = _host_inputs(**inputs)
    res = run_bass_kernel_spmd(nc, in_maps, core_ids=list(range(NCORES)))
    LAST_RESULTS = res
    # y wire layout is [BPC, CCH, NGRP, P, GW]; unpack to [BPC, C, N]
    out = np.concatenate(
        [
            np.asarray(r["y"]).transpose(0, 1, 3, 2, 4).reshape(BPC, C, N)
            for r in res.results
        ],
        axis=0,
    )
    return out.reshape(B, C, H, W).astype(np.float32)
